# revision 1
# baseline (speedup 1.0000x reference)
"""Causal self-attention TRN2 kernel: 8-way head-parallel (2 heads/core).

Layout strategy (per core c, heads h0=2c, h1=2c+1):
  - Host pre-transposes x -> xT [1024, 4096] (tokens b-major) and slices/permutes
    weights so q/k head dims are [32 evens | 32 odds] (de-interleaved RoPE).
  - QKV projection computes qT/kT/vT [128 (2 heads' dims), tok] via f32r
    matmuls accumulating over 8 c-chunks, in 1024-col blocks.
  - RoPE on qT/kT with partition-aligned ops (signed sin table + 32-row swap).
  - V transposed per 128-tok chunk into V_aug [tok 128, V(64) | ones(64)] so the
    AV matmul also produces the softmax denominator in psum rows 64:127.
  - Scores computed TRANSPOSED: S^T[k,q] chunks, k-chunks batched in pairs so
    one ACT exp covers 2 chunks (scale=1/8; |s|<~20 so no max subtraction);
    causal mask via gpsimd affine_select (multiplicative zero post-exp).
  - Single PSUM pool, phases interleaved per batch so attention/out-proj of
    batch 0 overlap QKV of batch 1.
  - out-proj: lhsT=outT chunk [128,128], rhs=woT [128,1024] -> partial [tok,1024]
  - Host sums 8 partials (the tensor-parallel all-reduce) and reshapes.
"""

import sys

if "/opt/trn_rl_repo" not in sys.path:
    sys.path.insert(0, "/opt/trn_rl_repo")

import numpy as np

import concourse.bass as bass
import concourse.tile as tile
from concourse import bacc, mybir
from concourse.masks import make_identity

F32 = mybir.dt.float32
F32R = mybir.dt.float32r
EXP = mybir.ActivationFunctionType.Exp

B, T, D, H, DH = 2, 2048, 1024, 16, 64
NCORES = 8
TOK = B * T          # 4096
QB = 512             # attention q-block (one psum bank wide)
KC = 128             # k chunk
EG = 2               # exp batch: k-chunks per ACT exp
NKC = T // KC        # 16 k-chunks per unit
NQB = T // QB        # 4 q-blocks per unit
PB = 1024            # QKV/out-proj column block
CPJ = D // 128       # 8 contraction chunks


def build_program():
    nc = bacc.Bacc("TRN2", target_bir_lowering=False, debug=False,
                   num_devices=NCORES)
    xT = nc.dram_tensor("xT", [D, TOK], F32R, kind="ExternalInput").ap()
    wq = nc.dram_tensor("wq", [D, 128], F32R, kind="ExternalInput").ap()
    wk = nc.dram_tensor("wk", [D, 128], F32R, kind="ExternalInput").ap()
    wv = nc.dram_tensor("wv", [D, 128], F32R, kind="ExternalInput").ap()
    woT = nc.dram_tensor("woT", [128, D], F32R, kind="ExternalInput").ap()
    cosT = nc.dram_tensor("cosT", [32, T], F32, kind="ExternalInput").ap()
    sinTp = nc.dram_tensor("sinTp", [32, T], F32, kind="ExternalInput").ap()
    sinTn = nc.dram_tensor("sinTn", [32, T], F32, kind="ExternalInput").ap()
    partial = nc.dram_tensor("partial", [TOK, D], F32, kind="ExternalOutput").ap()

    with tile.TileContext(nc) as tc:
        with tc.tile_pool(name="sb", bufs=1) as sb, \
             tc.tile_pool(name="ps", bufs=1, space="PSUM") as ps:
            # persistent SBUF tiles
            wq_sb = sb.tile([128, CPJ, 128], F32R, name="wq_sb", tag="wq_sb")
            wk_sb = sb.tile([128, CPJ, 128], F32R, name="wk_sb", tag="wk_sb")
            wv_sb = sb.tile([128, CPJ, 128], F32R, name="wv_sb", tag="wv_sb")
            woT_sb = sb.tile([128, D], F32R, name="woT_sb", tag="woT_sb")
            cos_full = sb.tile([128, T], F32, name="cos_full", tag="cos_full")
            sin_full = sb.tile([128, T], F32, name="sin_full", tag="sin_full")
            ident = sb.tile([64, 64], F32, name="ident", tag="ident")
            identB = sb.tile([128, 64], F32, name="identB", tag="identB")
            ones64 = sb.tile([128, 64], F32, name="ones64", tag="ones64")
            qT2 = sb.tile([128, TOK], F32R, name="qT2", tag="qT2")
            kT2 = sb.tile([128, TOK], F32R, name="kT2", tag="kT2")
            outT = sb.tile([128, TOK], F32R, name="outT", tag="outT")
            vaug = [sb.tile([128, NKC, 128], F32R, name=f"vaug{u}", tag=f"vaug{u}")
                    for u in range(4)]

            def emit_setup():
                make_identity(nc, ident[:], nomemset=False)
                nc.gpsimd.memset(identB[:], 0.0)
                nc.gpsimd.affine_select(out=identB[:], in_=identB[:],
                                        compare_op=mybir.AluOpType.not_equal,
                                        fill=1.0, base=-64, pattern=[[-1, 64]],
                                        channel_multiplier=1)
                nc.gpsimd.memset(ones64[:], 1.0)
                for blk in range(4):
                    nc.sync.dma_start(out=cos_full[blk * 32:(blk + 1) * 32, :],
                                      in_=cosT[:])
                nc.sync.dma_start(out=sin_full[0:32, :], in_=sinTp[:])
                nc.sync.dma_start(out=sin_full[32:64, :], in_=sinTn[:])
                nc.sync.dma_start(out=sin_full[64:96, :], in_=sinTp[:])
                nc.sync.dma_start(out=sin_full[96:128, :], in_=sinTn[:])
                nc.sync.dma_start(out=woT_sb[:], in_=woT[:])

            def emit_qkv_block(s):
                scol = s * PB
                tcol = (s % (T // PB)) * PB
                b = s // (T // PB)
                pss = {}
                for nm in ("q", "k", "v"):
                    pss[nm] = ps.tile([128, PB], F32, name=f"{nm}ps{s}",
                                      tag="big", bufs=3)
                for j in range(CPJ):
                    xts = sb.tile([128, PB], F32R, name=f"xts{s}_{j}",
                                  tag="xts", bufs=3)
                    nc.sync.dma_start(
                        out=xts[:], in_=xT[j * 128:(j + 1) * 128, scol:scol + PB])
                    if s == 0:      # interleave weight loads with first block
                        nc.sync.dma_start(out=wq_sb[:, j, :],
                                          in_=wq[j * 128:(j + 1) * 128, :])
                        nc.sync.dma_start(out=wk_sb[:, j, :],
                                          in_=wk[j * 128:(j + 1) * 128, :])
                        nc.sync.dma_start(out=wv_sb[:, j, :],
                                          in_=wv[j * 128:(j + 1) * 128, :])
                    st, sp = (j == 0), (j == CPJ - 1)
                    for nm, wsb in (("q", wq_sb), ("k", wk_sb), ("v", wv_sb)):
                        for hf in range(2):
                            nc.tensor.matmul(
                                pss[nm][:, hf * 512:(hf + 1) * 512],
                                wsb[:, j, :], xts[:, hf * 512:(hf + 1) * 512],
                                start=st, stop=sp)
                if s == 0:
                    emit_setup()

                raws = {}
                for nm in ("q", "k", "v"):
                    raw = sb.tile([128, PB], F32, name=f"raw{nm}{s}",
                                  tag=f"raw{nm}", bufs=2)
                    nc.scalar.copy(raw[:], pss[nm][:])
                    raws[nm] = raw
                for nm, dst in (("q", qT2), ("k", kT2)):
                    raw = raws[nm]
                    ra = sb.tile([128, PB], F32, name=f"ra{nm}{s}", tag="ropeA",
                                 bufs=2)
                    rs = sb.tile([128, PB], F32, name=f"rs{nm}{s}", tag="ropeS",
                                 bufs=2)
                    rw = sb.tile([128, PB], F32, name=f"rw{nm}{s}", tag="ropeW",
                                 bufs=2)
                    nc.vector.tensor_mul(ra[:], raw[:], cos_full[:, tcol:tcol + PB])
                    nc.vector.tensor_mul(rs[:], raw[:], sin_full[:, tcol:tcol + PB])
                    for blk in range(4):
                        src = (blk ^ 1) * 32
                        nc.sync.dma_start(out=rw[blk * 32:(blk + 1) * 32, :],
                                          in_=rs[src:src + 32, :])
                    nc.vector.tensor_add(dst[:, scol:scol + PB], ra[:], rw[:])

                vraw = raws["v"]
                for tp2 in range(PB // KC // 2):    # pairs of 128-tok chunks
                    for h in range(2):
                        u = b * 2 + h
                        ck0 = (s % (T // PB)) * (PB // KC) + 2 * tp2
                        tp = ps.tile([128, 2, 64], F32, name=f"tp{s}_{tp2}_{h}",
                                     tag="avtp", bufs=2)
                        idt = ident[:] if h == 0 else identB[64:128, :]
                        for pi in range(2):
                            tch = 2 * tp2 + pi
                            nc.tensor.transpose(
                                tp[:, pi, :],
                                vraw[h * 64:(h + 1) * 64, tch * KC:(tch + 1) * KC],
                                idt)
                        nc.vector.tensor_copy(vaug[u][:, ck0:ck0 + 2, 0:64], tp[:])

            def emit_attention_unit(u, op_after=False):
                b, h = u // 2, u % 2
                hr = h * 64
                tb = b * T
                for s4 in range(NQB):
                    qc = tb + s4 * QB
                    av = ps.tile([128, QB], F32, name=f"av{u}_{s4}", tag="avtp",
                                 bufs=2)
                    njc = (s4 + 1) * (QB // KC)
                    jgs = [list(range(g, min(g + EG, njc)))
                           for g in range(0, njc, EG)]
                    for jg in jgs:
                        ng = len(jg)
                        sps = ps.tile([128, EG, QB], F32,
                                      name=f"sps{u}_{s4}_{jg[0]}", tag="big",
                                      bufs=3)
                        for gi, j in enumerate(jg):
                            kc = tb + j * KC
                            nc.tensor.matmul(
                                sps[:, gi, :], kT2[hr:hr + 64, kc:kc + KC],
                                qT2[hr:hr + 64, qc:qc + QB],
                                start=True, stop=True)
                        pT = sb.tile([128, EG, QB], F32R,
                                     name=f"pT{u}_{s4}_{jg[0]}", tag="pT",
                                     bufs=4)
                        nc.scalar.activation(pT[:, 0:ng, :], sps[:, 0:ng, :],
                                             EXP, scale=0.125)
                        for gi, j in enumerate(jg):
                            off = j * KC - s4 * QB
                            if off >= -KC + 1:
                                nc.gpsimd.affine_select(
                                    out=pT[:, gi, :], in_=pT[:, gi, :],
                                    compare_op=mybir.AluOpType.is_ge,
                                    fill=0.0, base=-off, pattern=[[1, QB]],
                                    channel_multiplier=-1)
                        for gi, j in enumerate(jg):
                            nc.tensor.matmul(av[:], vaug[u][:, j, :],
                                             pT[:, gi, :], start=(j == 0),
                                             stop=(j == njc - 1))
                    rD = sb.tile([64, QB], F32, name=f"rD{u}_{s4}", tag="rD",
                                 bufs=2)
                    nc.vector.reciprocal(rD[:], av[64:128, :])
                    nc.vector.tensor_mul(outT[hr:hr + 64, qc:qc + QB],
                                         av[0:64, :], rD[:])
                    if op_after:
                        for mm in range(s4 * (QB // 128), (s4 + 1) * (QB // 128)):
                            emit_outproj_tile(b, mm)

            def emit_outproj_batch(b):
                for mm in range(T // 128):
                    emit_outproj_tile(b, mm)

            def emit_outproj_tile(b, mm):
                    col = b * T + mm * 128
                    ops = ps.tile([128, D], F32, name=f"ops{b}_{mm}", tag="big",
                                  bufs=3)
                    for hf in range(2):
                        nc.tensor.matmul(ops[:, hf * 512:(hf + 1) * 512],
                                         outT[:, col:col + 128],
                                         woT_sb[:, hf * 512:(hf + 1) * 512],
                                         start=True, stop=True)
                    osb = sb.tile([128, D], F32, name=f"osb{b}_{mm}", tag="osb",
                                  bufs=3)
                    nc.vector.tensor_copy(osb[:], ops[:])
                    nc.sync.dma_start(out=partial[col:col + 128, :], in_=osb[:])

            # interleaved emission: batch 0 attention overlaps batch 1 QKV
            emit_qkv_block(0)
            emit_qkv_block(1)
            for u in range(4):
                for ck in range(NKC):
                    nc.gpsimd.tensor_copy(out=vaug[u][:, ck, 64:128],
                                          in_=ones64[:])
            emit_attention_unit(0)
            emit_qkv_block(2)
            emit_attention_unit(1)
            emit_qkv_block(3)
            emit_attention_unit(2)
            emit_outproj_batch(0)
            emit_attention_unit(3)
            emit_outproj_batch(1)

    nc.compile()
    return nc


def prep_in_maps(x, rope_freqs, w_qkv, w_out):
    x = np.ascontiguousarray(x, dtype=np.float32)
    w_qkv = np.ascontiguousarray(w_qkv, dtype=np.float32)
    w_out = np.ascontiguousarray(w_out, dtype=np.float32)
    ang = np.asarray(rope_freqs, dtype=np.float64)
    cosT = np.ascontiguousarray(np.cos(ang).T.astype(np.float32))
    sinT = np.ascontiguousarray(np.sin(ang).T.astype(np.float32))
    sinTn = np.ascontiguousarray(-sinT)
    xT = np.ascontiguousarray(x.reshape(TOK, D).T)

    perm64 = np.concatenate([np.arange(0, DH, 2), np.arange(1, DH, 2)])
    in_maps = []
    for c in range(NCORES):
        h0, h1 = 2 * c, 2 * c + 1
        qk_rows = np.concatenate([h0 * DH + perm64, h1 * DH + perm64])
        v_rows = np.arange(h0 * DH, h0 * DH + 2 * DH)
        in_maps.append({
            "xT": xT,
            "wq": np.ascontiguousarray(w_qkv[qk_rows, :].T),
            "wk": np.ascontiguousarray(w_qkv[D + qk_rows, :].T),
            "wv": np.ascontiguousarray(w_qkv[2 * D + v_rows, :].T),
            "woT": np.ascontiguousarray(w_out[:, v_rows].T),
            "cosT": cosT, "sinTp": sinT, "sinTn": sinTn,
        })
    return in_maps


_CACHED = {}


def kernel(x, rope_freqs, w_qkv, w_out):
    from concourse.bass_utils import run_bass_kernel_spmd
    if "nc" not in _CACHED:
        _CACHED["nc"] = build_program()
    nc = _CACHED["nc"]
    in_maps = prep_in_maps(x, rope_freqs, w_qkv, w_out)
    res = run_bass_kernel_spmd(nc, in_maps, list(range(NCORES)))
    acc = np.zeros((TOK, D), dtype=np.float32)
    for r in res.results:
        acc += r["partial"]
    return acc.reshape(B, T, D)



# revision 10
# speedup vs baseline: 1.0988x; 1.0988x over previous
"""Causal self-attention TRN2 kernel: 8-way head-parallel (2 heads/core).

v4: all-bf16 dataflow, shifted-zipper schedule, depth-4 attention pipe.

Layout strategy (per core c, heads h0=2c, h1=2c+1):
  - Host pre-casts to bf16 and pre-transposes: xT [1024, 4096] (tokens
    b-major), weight images in SBUF layout ([128, 8*128] chunked), rope
    tables expanded to [128, T] with sign pattern baked in.
  - QKV: bf16 matmuls accumulate 8 c-chunks into [128, 512] PSUM tiles,
    one per q/k/v per 512-token halfblock.
  - RoPE: two DVE scalar_tensor_tensor muls straight from PSUM (no raw
    copy), partition swap via 2 strided SBUF->SBUF DMAs, gpsimd add.
  - V transposed per 128-tok chunk into vaug [tok, V(64)|ones(64)] so
    the AV matmul also produces the softmax denominator.
  - Attention per (batch, head) unit: q-blocks of QB=256; S^T chunks
    [128 k, 256 q] bf16; exp on ACT in EG=2 chunk groups (scale=1/8, no
    max subtraction); causal mask via gpsimd affine_select on the two
    diagonal chunks of each q-block. sps pool is 4 x 1-bank slots, so
    up to 4 score-groups are in flight against the exp latency chain.
  - Schedule: shifted zipper - step s emits attention (+out-proj) for
    halfblock s-1 interleaved with the QKV parts of halfblock s, units
    round-robined group-wise.
  - out-proj per 128-tok tile; partials stored bf16, 4 tiles per DMA.
  - Host sums 8 bf16 partials in f32 (the tensor-parallel all-reduce).
"""

import sys

if "/opt/trn_rl_repo" not in sys.path:
    sys.path.insert(0, "/opt/trn_rl_repo")

import numpy as np
import ml_dtypes

import concourse.bass as bass
import concourse.tile as tile
from concourse import bacc, mybir
from concourse.masks import make_identity

F32 = mybir.dt.float32
BF16 = mybir.dt.bfloat16
EXP = mybir.ActivationFunctionType.Exp
MUL = mybir.AluOpType.mult

B, T, D, H, DH = 2, 2048, 1024, 16, 64
NCORES = 8
TOK = B * T          # 4096
HB = 512             # QKV halfblock (tokens)
NHB = TOK // HB      # 8
QB = 256             # attention q-block
KC = 128             # k chunk
EG = 2               # exp batch: k-chunks per ACT exp
CPJ = D // 128       # 8 contraction chunks


def build_program():
    nc = bacc.Bacc("TRN2", target_bir_lowering=False, debug=False,
                   num_devices=NCORES)
    xT = nc.dram_tensor("xT", [D, TOK], BF16, kind="ExternalInput").ap()
    wq = nc.dram_tensor("wq", [128, D], BF16, kind="ExternalInput").ap()
    wk = nc.dram_tensor("wk", [128, D], BF16, kind="ExternalInput").ap()
    wv = nc.dram_tensor("wv", [128, D], BF16, kind="ExternalInput").ap()
    woT = nc.dram_tensor("woT", [128, D], BF16, kind="ExternalInput").ap()
    cosF = nc.dram_tensor("cosF", [128, T], BF16, kind="ExternalInput").ap()
    sinF = nc.dram_tensor("sinF", [128, T], BF16, kind="ExternalInput").ap()
    partial = nc.dram_tensor("partial", [TOK, D], BF16,
                             kind="ExternalOutput").ap()

    with tile.TileContext(nc) as tc:
        with tc.tile_pool(name="sb", bufs=1) as sb, \
             tc.tile_pool(name="ps", bufs=1, space="PSUM") as ps:
            # persistent SBUF tiles
            wq_sb = sb.tile([128, CPJ, 128], BF16, name="wq_sb", tag="wq_sb")
            wk_sb = sb.tile([128, CPJ, 128], BF16, name="wk_sb", tag="wk_sb")
            wv_sb = sb.tile([128, CPJ, 128], BF16, name="wv_sb", tag="wv_sb")
            woT_sb = sb.tile([128, D], BF16, name="woT_sb", tag="woT_sb")
            cos_sb = sb.tile([128, T], BF16, name="cos_sb", tag="cos_sb")
            sin_sb = sb.tile([128, T], BF16, name="sin_sb", tag="sin_sb")
            ident = sb.tile([64, 64], F32, name="ident", tag="ident")
            identB = sb.tile([128, 64], F32, name="identB", tag="identB")
            qT2 = sb.tile([128, TOK], BF16, name="qT2", tag="qT2")
            kT2 = sb.tile([128, TOK], BF16, name="kT2", tag="kT2")
            outT = sb.tile([128, TOK], BF16, name="outT", tag="outT")
            vaug = [sb.tile([128, T // KC, 128], BF16, name=f"vaug{u}",
                            tag=f"vaug{u}") for u in range(4)]

            def emit_setup():
                make_identity(nc, ident[:], nomemset=False)
                nc.gpsimd.memset(identB[:], 0.0)
                nc.gpsimd.affine_select(out=identB[:], in_=identB[:],
                                        compare_op=mybir.AluOpType.not_equal,
                                        fill=1.0, base=-64, pattern=[[-1, 64]],
                                        channel_multiplier=1)
                for u in range(4):
                    nc.gpsimd.memset(vaug[u][:, :, 64:128], 1.0)
                nc.sync.dma_start(out=wq_sb[:], in_=wq[:])

            def emit_setup_late():
                nc.sync.dma_start(out=cos_sb[:], in_=cosF[:])
                nc.sync.dma_start(out=sin_sb[:], in_=sinF[:])
                nc.sync.dma_start(out=woT_sb[:], in_=woT[:])

            xts_tiles = {}

            def emit_xts(hb):
                tok0 = hb * HB
                xts = sb.tile([128, CPJ, HB], BF16, name=f"xts{hb}",
                              tag="xts", bufs=3)
                src = xT[:, tok0:tok0 + HB].rearrange("(j p) t -> p j t",
                                                      p=128)
                if hb <= 1:
                    # split so the first fill matmuls start sooner
                    nc.sync.dma_start(out=xts[:, 0:4, :], in_=src[:, 0:4, :])
                    if hb == 0:
                        nc.sync.dma_start(out=wk_sb[:], in_=wk[:])
                    nc.sync.dma_start(out=xts[:, 4:8, :], in_=src[:, 4:8, :])
                    if hb == 0:
                        nc.sync.dma_start(out=wv_sb[:], in_=wv[:])
                        emit_setup_late()
                else:
                    nc.sync.dma_start(out=xts[:], in_=src)
                xts_tiles[hb] = xts

            def emit_qkv_qk(hb, nm):
                tok0 = hb * HB
                tloc = (hb % (T // HB)) * HB
                xts = xts_tiles[hb]
                wsb, dst = ((wq_sb, qT2) if nm == "q" else (wk_sb, kT2))
                acc = ps.tile([128, HB], F32, name=f"acc{nm}{hb}",
                              tag="wk", bufs=2)
                for j in range(CPJ):
                    nc.tensor.matmul(acc[:], wsb[:, j, :], xts[:, j, :],
                                     start=(j == 0), stop=(j == CPJ - 1))
                ra = sb.tile([128, HB], BF16, name=f"ra{nm}{hb}",
                             tag="ropeA", bufs=4)
                rs = sb.tile([128, HB], BF16, name=f"rs{nm}{hb}",
                             tag="ropeS", bufs=4)
                rw = sb.tile([128, HB], BF16, name=f"rw{nm}{hb}",
                             tag="ropeW", bufs=4)
                # ra = (acc * 1) * cos ; rs = (acc * 1) * sin  (psum read)
                nc.vector.scalar_tensor_tensor(
                    out=ra[:], in0=acc[:], scalar=1.0,
                    in1=cos_sb[:, tloc:tloc + HB], op0=MUL, op1=MUL)
                nc.vector.scalar_tensor_tensor(
                    out=rs[:], in0=acc[:], scalar=1.0,
                    in1=sin_sb[:, tloc:tloc + HB], op0=MUL, op1=MUL)
                rs4 = rs[:].rearrange("(c a p) t -> c a p t", c=2, a=2)
                rw4 = rw[:].rearrange("(c a p) t -> c a p t", c=2, a=2)
                nc.sync.dma_start(out=rw4[:, 0], in_=rs4[:, 1])
                nc.sync.dma_start(out=rw4[:, 1], in_=rs4[:, 0])
                nc.gpsimd.tensor_add(dst[:, tok0:tok0 + HB], ra[:], rw[:])

            def emit_qkv_v(hb):
                b = hb // (T // HB)
                xts = xts_tiles[hb]
                accv = ps.tile([128, HB], F32, name=f"accv{hb}", tag="wk",
                               bufs=2)
                for j in range(CPJ):
                    nc.tensor.matmul(accv[:], wv_sb[:, j, :], xts[:, j, :],
                                     start=(j == 0), stop=(j == CPJ - 1))
                vraw = sb.tile([128, HB], F32, name=f"vraw{hb}", tag="rawv",
                               bufs=2)
                nc.scalar.copy(vraw[:], accv[:])
                for tp2 in range(2):
                    for h in range(2):
                        u = b * 2 + h
                        ck0 = (hb % (T // HB)) * (HB // KC) + 2 * tp2
                        tp = ps.tile([128, 2, 64], F32,
                                     name=f"tp{hb}_{tp2}_{h}", tag="sps",
                                     bufs=4)
                        idt = ident[:] if h == 0 else identB[64:128, :]
                        for pi in range(2):
                            tch = 2 * tp2 + pi
                            nc.tensor.transpose(
                                tp[:, pi, :],
                                vraw[h * 64:(h + 1) * 64,
                                     tch * KC:(tch + 1) * KC],
                                idt)
                        nc.vector.tensor_copy(vaug[u][:, ck0:ck0 + 2, 0:64],
                                              tp[:])

            def emit_attention_pair(b, m):
                """Both units' q-block m of batch b, groups round-robined."""
                tb = b * T
                qc = tb + m * QB
                njc = (m + 1) * (QB // KC)
                jgs = [list(range(g, min(g + EG, njc)))
                       for g in range(0, njc, EG)]
                avs = {}
                for h in range(2):
                    avs[h] = ps.tile([128, QB], F32, name=f"av{b}_{h}_{m}",
                                     tag="av", bufs=2)
                pending = []

                def flush_av():
                    h, jg, pT = pending.pop(0)
                    u = 2 * b + h
                    av = avs[h]
                    for gi, j in enumerate(jg):
                        nc.tensor.matmul(av[:], vaug[u][:, j, :],
                                         pT[:, gi, :], start=(j == 0),
                                         stop=(j == njc - 1))

                for jg in jgs:
                    ng = len(jg)
                    for h in range(2):
                        u, hr = 2 * b + h, h * 64
                        sps = ps.tile([128, EG, QB], F32,
                                      name=f"sps{u}_{m}_{jg[0]}", tag="sps",
                                      bufs=4)
                        for gi, j in enumerate(jg):
                            kc = tb + j * KC
                            nc.tensor.matmul(
                                sps[:, gi, :], kT2[hr:hr + 64, kc:kc + KC],
                                qT2[hr:hr + 64, qc:qc + QB],
                                start=True, stop=True)
                        pT = sb.tile([128, EG, QB], BF16,
                                     name=f"pT{u}_{m}_{jg[0]}", tag="pT",
                                     bufs=8)
                        nc.scalar.activation(pT[:, 0:ng, :], sps[:, 0:ng, :],
                                             EXP, scale=0.125)
                        for gi, j in enumerate(jg):
                            off = j * KC - m * QB
                            if off >= -KC + 1:
                                nc.gpsimd.affine_select(
                                    out=pT[:, gi, :], in_=pT[:, gi, :],
                                    compare_op=mybir.AluOpType.is_ge,
                                    fill=0.0, base=-off, pattern=[[1, QB]],
                                    channel_multiplier=-1)
                        pending.append((h, jg, pT))
                        # software-pipeline: emit AV one group-pair behind
                        if len(pending) > 2:
                            flush_av()
                while pending:
                    flush_av()
                return avs

            def emit_norm_pair(b, m, avs):
                qc = b * T + m * QB
                for h in range(2):
                    u, hr = 2 * b + h, h * 64
                    av = avs[h]
                    rD = sb.tile([64, QB], F32, name=f"rD{u}_{m}", tag="rD",
                                 bufs=3)
                    nc.vector.reciprocal(rD[:], av[64:128, :])
                    nc.vector.tensor_mul(outT[hr:hr + 64, qc:qc + QB],
                                         av[0:64, :], rD[:])

            osb_state = {}

            def emit_op_tile(b, mm):
                col = b * T + mm * 128
                grp = mm // 4
                if (b, grp) not in osb_state:
                    osb_state[(b, grp)] = sb.tile(
                        [128, 4, D], BF16, name=f"osb{b}_{grp}", tag="osb",
                        bufs=2)
                osb = osb_state[(b, grp)]
                for hf in range(2):
                    ops = ps.tile([128, 512], F32, name=f"ops{b}_{mm}_{hf}",
                                  tag="wk", bufs=2)
                    nc.tensor.matmul(ops[:], outT[:, col:col + 128],
                                     woT_sb[:, hf * 512:(hf + 1) * 512],
                                     start=True, stop=True)
                    nc.vector.tensor_copy(
                        osb[:, mm % 4, hf * 512:(hf + 1) * 512], ops[:])
                if mm % 4 == 3:
                    base = b * T + grp * 512
                    nc.sync.dma_start(
                        out=partial[base:base + 512, :].rearrange(
                            "(i p) c -> p i c", p=128),
                        in_=osb[:])

            # shifted zipper: step s = attention(hb s-1) zipped with QKV(hb s)
            # out-proj for hb s-1 is deferred to step s+1 so its DVE copies
            # and the norm chain never head-of-line-block feed-forward work.
            emit_setup()
            emit_xts(0)
            emit_qkv_qk(0, "q")
            emit_qkv_qk(0, "k")
            emit_qkv_v(0)
            for s in range(1, NHB + 2):
                a = s - 1            # attention halfblock
                o = s - 2            # out-proj halfblock
                if a < NHB:
                    ba, la = a // (T // HB), a % (T // HB)
                    if s < NHB:
                        emit_xts(s)
                    avs0 = emit_attention_pair(ba, 2 * la)
                    if s < NHB:
                        emit_qkv_qk(s, "q")
                    avs1 = emit_attention_pair(ba, 2 * la + 1)
                    if s < NHB:
                        emit_qkv_qk(s, "k")
                        emit_qkv_v(s)
                    emit_norm_pair(ba, 2 * la, avs0)
                    emit_norm_pair(ba, 2 * la + 1, avs1)
                if o >= 0:
                    bo, lo = o // (T // HB), o % (T // HB)
                    for mm in range(4 * lo, 4 * lo + 4):
                        emit_op_tile(bo, mm)

    nc.compile()
    return nc


def prep_in_maps(x, rope_freqs, w_qkv, w_out):
    bf = ml_dtypes.bfloat16
    x = np.ascontiguousarray(x, dtype=np.float32)
    w_qkv = np.asarray(w_qkv, dtype=np.float32)
    w_out = np.asarray(w_out, dtype=np.float32)
    ang = np.asarray(rope_freqs, dtype=np.float64)
    cosT = np.cos(ang).T            # [32, T]
    sinT = np.sin(ang).T
    cosF = np.ascontiguousarray(
        np.tile(cosT, (4, 1)).astype(bf))                       # [128, T]
    sinF = np.ascontiguousarray(
        np.concatenate([sinT, -sinT, sinT, -sinT]).astype(bf))  # [128, T]
    xT = np.ascontiguousarray(x.reshape(TOK, D).T.astype(bf))

    perm64 = np.concatenate([np.arange(0, DH, 2), np.arange(1, DH, 2)])

    def w_img(wsel):
        # wsel [128 outdims, D] -> SBUF image [128, 8*128]:
        # img[p, 128j+col] = wsel[col, 128j+p]
        return np.ascontiguousarray(
            wsel.T.reshape(CPJ, 128, 128).transpose(1, 0, 2)
            .reshape(128, D).astype(bf))

    in_maps = []
    for c in range(NCORES):
        h0 = 2 * c
        qk_rows = np.concatenate([h0 * DH + perm64, (h0 + 1) * DH + perm64])
        v_rows = np.arange(h0 * DH, h0 * DH + 2 * DH)
        in_maps.append({
            "xT": xT,
            "wq": w_img(w_qkv[qk_rows, :]),
            "wk": w_img(w_qkv[D + qk_rows, :]),
            "wv": w_img(w_qkv[2 * D + v_rows, :]),
            "woT": np.ascontiguousarray(w_out[:, v_rows].T.astype(bf)),
            "cosF": cosF, "sinF": sinF,
        })
    return in_maps


_CACHED = {}


def kernel(x, rope_freqs, w_qkv, w_out):
    from concourse.bass_utils import run_bass_kernel_spmd
    if "nc" not in _CACHED:
        _CACHED["nc"] = build_program()
    nc = _CACHED["nc"]
    in_maps = prep_in_maps(x, rope_freqs, w_qkv, w_out)
    res = run_bass_kernel_spmd(nc, in_maps, list(range(NCORES)))
    acc = np.zeros((TOK, D), dtype=np.float32)
    for r in res.results:
        acc += np.asarray(r["partial"], dtype=np.float32)
    return acc.reshape(B, T, D)
